# revision 35
# baseline (speedup 1.0000x reference)
"""Trainium2 Bass kernel for nn_BasicConvolutionBlock (gather-GEMM sparse conv + BN + ReLU).

Math (see reference): for each of K=27 kernel offsets,
    conv += (feats[nbr_idx[k]] * mask[k,:,None]) @ W[k]
then train-mode BatchNorm over the N axis (global mean/var per channel) + ReLU.

Distribution: voxel dim N sharded over 8 cores (data parallel). feats table and
weights replicated to every core; each core gathers its shard's neighbors
locally via indirect DMA. BatchNorm stats are all-reduced across cores.

Perf notes (from HW traces): the bottleneck is the Pool/SWDGE engine --
each indirect gather instruction carries exactly ONE row offset per
partition (128 descriptors; multi-offset encodings are unimplemented in
the DGE: indirection is pinned to the source's slowest dim and walrus
only lowers the 2-dim dest form -- HW-probed), and costs ~1.16us of
serialized descriptor-generation time. 196 subtiles x 26 non-center
offsets = 5096 instructions ~= 5.9ms Pool time; everything else hides
under it. Optimizations vs the first working version (9.0ms -> ~6ms):
  - bf16 pipeline (host-converted feats/weights): halves gather bytes,
    G memset cost (DVE 2x), PE transpose cycles (1 vs 2 cyc/row)
  - center k-plane (identity map, contiguous) fetched via HWDGE (sync
    engine), off the Pool engine; collective staging DMAs also on sync
  - masked taps OOB-skipped via bounds reg (cheaper in SWDGE gen + SDMA
    than gathering a zero row: dummies beat real descriptors, measured)
  - G buffers pre-zeroed 2 tiles AHEAD so the gather stream never waits
    on the DVE FIFO (memset would otherwise queue behind PSUM->SBUF
    copies that wait on PE transposes, coupling Pool to PE each tile)
  - PSUM->SBUF rhs copies alternate DVE/ACT (both 1 PSUM input max)
Accumulation stays f32 in PSUM; BN stats are computed in f32.

Per-core pipeline:
  1. masked index fold: idx' = mask ? idx : N+1  (> bounds -> skipped, G
     pre-zeroed so masked taps contribute 0)
  2. per 512-row tile: 4x26 per-(subtile,k) indirect gathers into
     [128p, 4a, 27k, 64c] bf16 (per-tile queue blocks); per k-pair:
     PE-transpose subtiles to [128kc, 512pt] PSUM, DVE/ACT copy to SBUF
     bf16, PE matmul accumulating into PSUM [64, 512]
  3. per-tile partial stats (sum / sumsq on DVE); conv kept in SBUF bf16
  4. AllReduce [64,2] stats -> scale/shift; ACT fused affine+ReLU;
     PE transpose back to row-major; DMA out.
"""

import os
import sys

sys.path.insert(0, "/opt/trn_rl_repo")

import numpy as np
import ml_dtypes


def _install_ntff_hook_module():
    """Provide antenv.axon_hooks (NTFF profiling under axon) if the image
    lacks it, so run_bass_kernel_spmd(trace=True) can report exec_time_ns."""
    import importlib
    try:
        importlib.import_module("antenv.axon_hooks")
        return
    except ImportError:
        pass
    import contextlib
    import ctypes
    import types

    so_path = "/opt/axon/libaxon_pjrt.so"
    mod = types.ModuleType("antenv.axon_hooks")
    state = {"hook": None, "tried": False}

    def set_axon_ntff_profile_hook(hook):
        state["hook"] = hook

    def _build_hook():
        if not os.path.exists(so_path):
            return None
        lib = ctypes.CDLL(so_path)
        if not hasattr(lib, "axon_start_nrt_profile"):
            return None
        lib.axon_start_nrt_profile.argtypes = [
            ctypes.POINTER(ctypes.c_int64), ctypes.c_size_t]
        lib.axon_start_nrt_profile.restype = ctypes.c_int64
        lib.axon_stop_nrt_profile.argtypes = [ctypes.c_char_p]
        lib.axon_stop_nrt_profile.restype = ctypes.c_int64

        @contextlib.contextmanager
        def _hook(output_dir, device_ids):
            import jax
            jax.devices()
            if device_ids:
                ids = (ctypes.c_int64 * len(device_ids))(*device_ids)
                rc = lib.axon_start_nrt_profile(ids, len(device_ids))
            else:
                rc = lib.axon_start_nrt_profile(None, 0)
            if rc != 0:
                raise RuntimeError(f"axon_start_nrt_profile rc={rc}")
            try:
                yield
            finally:
                n = lib.axon_stop_nrt_profile(str(output_dir).encode())
                print(f"ntff profile: {n} file(s) -> {output_dir}",
                      file=sys.stderr)

        return _hook

    def get_axon_ntff_profile_hook():
        if state["hook"] is None and not state["tried"]:
            state["tried"] = True
            state["hook"] = _build_hook()
        return state["hook"]

    mod.set_axon_ntff_profile_hook = set_axon_ntff_profile_hook
    mod.get_axon_ntff_profile_hook = get_axon_ntff_profile_hook
    sys.modules["antenv.axon_hooks"] = mod


_install_ntff_hook_module()

import concourse.bass as bass
import concourse.bacc as bacc
import concourse.tile as tile
from concourse import mybir
from concourse.bass_utils import run_bass_kernel_spmd
from concourse.masks import make_identity

F32 = mybir.dt.float32
BF16 = mybir.dt.bfloat16
I32 = mybir.dt.int32
BF16_NP = np.dtype(ml_dtypes.bfloat16)


def _indirect_gather_q(nc, out_ap, in_ap, offset_ap, queue: str,
                       bounds_reg=None):
    """bass.indirect_dma_start (gather form) with a selectable SWDGE queue and
    an optional pre-made bounds register (indices > bound are skipped).

    The offset AP may carry many offsets per partition. The DGE pins the
    indirection dimension to the SOURCE tensor's slowest dim and fetches a
    new index each time the lockstep iteration crosses it, so the AP
    contract for an M-offsets-per-partition gather is:
      src:    [M, elem]   (outer num_elem = per-channel index count; its
                           stride is discarded -- the fetched index * coef
                           addresses the row)
      dest:   [128, M, elem]  (NOT flattened: a merged free dim becomes one
                           index + M*elem contiguous bytes)
      offset: [128, M] int32 (4B index walk per channel)
    This amortizes the ~1us SWDGE fixed cost over M*128 descriptors."""
    gp = nc.gpsimd
    table_rows = in_ap.shape[0]
    if len(out_ap.shape) > 2:
        m_per_ch = 1
        for s in out_ap.shape[1:-1]:
            m_per_ch *= s
        out_ap = out_ap.rearrange(
            f"p {' '.join(chr(97 + i) for i in range(len(out_ap.shape) - 2))} c"
            f" -> p ({' '.join(chr(97 + i) for i in range(len(out_ap.shape) - 2))}) c"
        )
        assert out_ap.shape == (128, m_per_ch, in_ap.shape[-1])
    offset_ap = offset_ap.opt()
    out_l = gp.lower_ap_dma(out_ap, for_indirect_dma=True)
    in_l = gp.lower_ap_dma(in_ap, for_indirect_dma=True)
    assert len(in_l) == 1 and len(out_l) == 1
    off_l = gp.lower_ap_dma(offset_ap)
    assert len(off_l) == 1
    in_l.append(off_l[0])
    ap_shape = in_ap.shape
    coef = 1
    for i in range(1, len(ap_shape)):
        coef *= ap_shape[i]
    in_l[0].dynamic_ap_info = mybir.DynamicAccessPatternInfo(
        c=0,
        actual_ap=out_ap.ap,
        indirect_dim_max_index=table_rows,
        offset_expr=[
            mybir.DynamicAccessPatternOffsetExpr(
                coef=coef,
                aff_expr=mybir.DynamicAccessPatternOffsetExprAffExpr(
                    kind="IndirectArgId", arg_id=1,
                ),
            )
        ],
    )
    if bounds_reg is not None:
        in_l = in_l + [gp.lower_val_access(bounds_reg)]
    return gp.add_instruction(
        mybir.InstDMACopy(
            name=nc.get_next_instruction_name(),
            queue=queue,
            mode="Copy",
            ins=in_l,
            outs=out_l,
            oob_is_err=False,
            cce_op=mybir.AluOpType.bypass,
        )
    )


class Cfg:
    def __init__(self, n=200000, c=64, k=27, n_cores=8, tile_rows=512,
                 eps=1e-5, n_queues=4, act_copies=True, zero_row=False):
        # zero_row: masked taps read the table's zero row (row n) instead of
        # being OOB-skipped -- drops the bounds check and the G memset at the
        # cost of extra (DRAM-row-hot) gather traffic.
        self.zero_row = zero_row
        assert n % n_cores == 0
        self.n, self.c, self.k, self.n_cores = n, c, k, n_cores
        self.eps = eps
        self.shard = n // n_cores
        self.nsub = (self.shard + 127) // 128          # 128-row subtiles
        self.shard_pad = self.nsub * 128
        self.tile_rows = tile_rows                     # rows per PSUM tile
        self.a_per_tile = tile_rows // 128             # subtiles per tile
        assert self.nsub % self.a_per_tile == 0
        self.nt = self.shard_pad // tile_rows          # tiles per core
        self.npair = (k + 1) // 2                      # last pair may be single
        self.n_queues = n_queues
        self.act_copies = act_copies                   # PSUM->SBUF copies split DVE/ACT
        self.table_rows = n + 1                        # + zero row (unused; bounds pad)


def build_kernel(cfg: Cfg):
    nc = bacc.Bacc("TRN2", target_bir_lowering=False, debug=False,
                   num_devices=cfg.n_cores, num_swdge_queues=cfg.n_queues)
    C, K = cfg.c, cfg.k
    TR, AT = cfg.tile_rows, cfg.a_per_tile
    KP = K

    feats = nc.dram_tensor("feats", [cfg.table_rows, C], BF16, kind="ExternalInput")
    wflat = nc.dram_tensor("wflat", [K * C, C], BF16, kind="ExternalInput")
    gamma = nc.dram_tensor("gamma", [C, 1], F32, kind="ExternalInput")
    beta = nc.dram_tensor("beta", [C, 1], F32, kind="ExternalInput")
    # host-transposed indices/mask: [128, nsub, k] with (p, a, k) = idx[k, a*128+p]
    idxT = nc.dram_tensor("idxT", [128, cfg.nsub * K], I32, kind="ExternalInput")
    maskT = nc.dram_tensor("maskT", [128, cfg.nsub * K], I32, kind="ExternalInput")
    # center k-plane of the gather is the identity map over the core's own
    # shard -- a contiguous read that HWDGE (sync engine) can do, keeping
    # those 196 transfers off the serialized Pool/SWDGE engine.
    center = nc.dram_tensor("center", [cfg.shard_pad, C], BF16, kind="ExternalInput")
    outp = nc.dram_tensor("out", [cfg.shard_pad, C], F32, kind="ExternalOutput")

    with tile.TileContext(nc) as tc:
        with (
            tc.tile_pool(name="singles", bufs=1) as singles,
            tc.tile_pool(name="gpool", bufs=1) as gpool,
            tc.tile_pool(name="trp", bufs=3, space="PSUM") as trp,
            tc.tile_pool(name="rhsp", bufs=4) as rhsp,
            tc.tile_pool(name="pacc", bufs=2, space="PSUM") as pacc,
            tc.tile_pool(name="pout", bufs=1, space="PSUM") as pout,
            tc.tile_pool(name="outsb", bufs=3) as outsb,
            tc.tile_pool(name="small", bufs=4) as small,
            tc.tile_pool(name="dram", bufs=1, space="DRAM") as dram,
        ):
            # ---------- constants ----------
            ident = singles.tile([128, 128], BF16)
            make_identity(nc, ident[:])
            identf = singles.tile([C, C], F32)
            make_identity(nc, identf[:])

            w_sb = singles.tile([128, cfg.npair * C], BF16)
            npair_full = K // 2  # pairs with both k's real
            nc.vector.memset(w_sb[:], 0.0)
            nc.sync.dma_start(
                out=w_sb[:, : npair_full * C].rearrange("p (j c) -> p j c", j=npair_full),
                in_=wflat[: npair_full * 128, :].rearrange("(j p) c -> p j c", p=128),
            )
            if K % 2:
                # trailing single k in the top 64 partitions of the last slot
                nc.sync.dma_start(
                    out=w_sb[:C, npair_full * C:(npair_full + 1) * C],
                    in_=wflat[(K - 1) * C: K * C, :],
                )

            gam = singles.tile([C, 1], F32)
            bet = singles.tile([C, 1], F32)
            nc.sync.dma_start(out=gam[:], in_=gamma[:])
            nc.sync.dma_start(out=bet[:], in_=beta[:])
            epst = singles.tile([C, 1], F32)
            nc.vector.memset(epst[:], cfg.eps)

            # ---------- masked index fold ----------
            # idx' = mask ? idx : (n+1 | n). With bounds check (bound = n)
            # n+1 entries are skipped by the DGE and the pre-zeroed G supplies
            # the zeros; in zero_row mode they read the zero row n directly.
            idx_sb = singles.tile([128, cfg.nsub, KP], I32)
            nc.vector.memset(idx_sb[:], cfg.n if cfg.zero_row else cfg.n + 1)
            with tc.tile_pool(name="idxstage", bufs=1) as stage:
                idx_raw = stage.tile([128, cfg.nsub, KP], I32)
                msk_raw = stage.tile([128, cfg.nsub, KP], I32)
                nc.sync.dma_start(out=idx_raw[:],
                                  in_=idxT[:].rearrange("p (a k) -> p a k", k=K))
                nc.sync.dma_start(out=msk_raw[:],
                                  in_=maskT[:].rearrange("p (a k) -> p a k", k=K))
                nc.vector.copy_predicated(
                    out=idx_sb[:], mask=msk_raw[:], data=idx_raw[:]
                )

            # allocated after the idx staging pool is freed (SBUF peak)
            conv_rm = singles.tile([128, cfg.nsub * C], BF16)

            conv_sb = singles.tile([C, cfg.shard_pad], BF16)
            stats_s = singles.tile([C, cfg.nt], F32)
            stats_q = singles.tile([C, cfg.nt], F32)

            # ---------- main conv loop ----------
            # HW constraint (probed): one indirect DMA consumes exactly one
            # row offset per partition -- 128 descriptors per instruction.
            bc_reg = None if cfg.zero_row else nc.gpsimd.to_reg(cfg.n)
            center_k = K // 2
            # Pre-zero G buffers AHEAD of use: the memset for tile t+lead is
            # emitted before tile t's PSUM->SBUF copies so it never queues
            # behind them in the DVE FIFO (which would couple the Pool gather
            # stream to PE transpose latency every tile).
            lead = 1
            gq = []
            for i in range(min(lead, cfg.nt)):
                Gn = gpool.tile([128, AT, KP, C], BF16, tag=f"g{i}")
                if not cfg.zero_row:
                    nc.vector.memset(Gn[:], 0.0)
                gq.append(Gn)
            for t in range(cfg.nt):
                G = gq.pop(0)
                if t + lead < cfg.nt:
                    Gn = gpool.tile([128, AT, KP, C], BF16, tag=f"g{(t + lead) % (lead + 2)}")
                    if not cfg.zero_row:
                        nc.vector.memset(Gn[:], 0.0)
                    gq.append(Gn)
                nc.sync.dma_start(
                    out=G[:, :, center_k, :],
                    in_=center[t * TR:(t + 1) * TR, :].rearrange(
                        "(s p) c -> p s c", p=128),
                )
                for s in range(AT):
                    a = t * AT + s
                    for k in range(KP):
                        if k == center_k:
                            continue
                        # one queue per tile (blocked): consecutive Pool
                        # instructions share the SWDGE ring context; drain
                        # still overlaps across tiles on rotating queues
                        q = t % cfg.n_queues
                        _indirect_gather_q(
                            nc,
                            out_ap=G[:, s, k, :],
                            in_ap=feats[:],
                            offset_ap=idx_sb[:, a, k:k + 1],
                            queue=f"qPoolDynamic{q or ''}",
                            bounds_reg=bc_reg,
                        )

                acc = pacc.tile([C, TR], F32)
                for j in range(cfg.npair):
                    single = (j == cfg.npair - 1) and (K % 2 == 1)
                    np_ = C if single else 2 * C
                    ptr = trp.tile([128, TR], BF16)
                    for s in range(AT):
                        nc.tensor.transpose(
                            out=ptr[:np_, s * 128:(s + 1) * 128],
                            in_=G[:, s, 2 * j:2 * j + (1 if single else 2), :],
                            identity=ident[:],
                        )
                    rhs = rhsp.tile([128, TR], BF16)
                    if cfg.act_copies and (j % 2 == 1):
                        nc.scalar.activation(
                            out=rhs[:np_, :], in_=ptr[:np_, :],
                            func=mybir.ActivationFunctionType.Copy,
                        )
                    else:
                        nc.vector.tensor_copy(out=rhs[:np_, :], in_=ptr[:np_, :])
                    nc.tensor.matmul(
                        out=acc[:],
                        lhsT=w_sb[:np_, j * C:(j + 1) * C],
                        rhs=rhs[:np_, :],
                        start=(j == 0),
                        stop=(j == cfg.npair - 1),
                    )

                # partial BN stats (all on DVE: keep ACT's function table on
                # Copy all run) + conv store
                nc.vector.reduce_sum(
                    out=stats_s[:, t:t + 1], in_=acc[:], axis=mybir.AxisListType.X
                )
                cs = conv_sb[:, t * TR:(t + 1) * TR]
                nc.vector.tensor_copy(out=cs, in_=acc[:])
                # sumsq from the SBUF bf16 copy (walrus allows only one PSUM
                # input per DVE op); consistent with the normalized values
                sq = small.tile([C, TR], F32)
                nc.vector.tensor_tensor(out=sq[:], in0=cs, in1=cs,
                                        op=mybir.AluOpType.mult)
                nc.vector.reduce_sum(
                    out=stats_q[:, t:t + 1], in_=sq[:], axis=mybir.AxisListType.X
                )

                # transpose UNNORMALIZED conv to row-major now, hidden under
                # the Pool gather stream; the final affine+ReLU then runs on
                # row-major data so the post-collective tail stays tiny
                po = pout.tile([128, AT * C], BF16)
                for s in range(AT):
                    nc.tensor.transpose(
                        out=po[:, s * C:(s + 1) * C],
                        in_=cs[:, s * 128:(s + 1) * 128],
                        identity=ident[:C, :C],
                    )
                nc.vector.tensor_copy(
                    out=conv_rm[:, t * AT * C:(t + 1) * AT * C], in_=po[:]
                )

            # ---------- global BN stats (AllReduce) ----------
            sums = small.tile([C, 2], F32)
            nc.vector.reduce_sum(out=sums[:, 0:1], in_=stats_s[:], axis=mybir.AxisListType.X)
            nc.vector.reduce_sum(out=sums[:, 1:2], in_=stats_q[:], axis=mybir.AxisListType.X)
            cc_in = dram.tile([C, 2], F32)
            cc_out = dram.tile([C, 2], F32)
            nc.sync.dma_start(out=cc_in[:], in_=sums[:])
            nc.gpsimd.collective_compute(
                "AllReduce",
                mybir.AluOpType.add,
                replica_groups=[list(range(cfg.n_cores))],
                ins=[cc_in.opt()],
                outs=[cc_out.opt()],
            )
            gsum = small.tile([C, 2], F32)
            nc.sync.dma_start(out=gsum[:], in_=cc_out[:])

            mean = small.tile([C, 1], F32)
            ex2 = small.tile([C, 1], F32)
            nc.scalar.mul(out=mean[:], in_=gsum[:, 0:1], mul=1.0 / cfg.n)
            nc.scalar.mul(out=ex2[:], in_=gsum[:, 1:2], mul=1.0 / cfg.n)
            var = small.tile([C, 1], F32)
            nc.vector.tensor_tensor(out=var[:], in0=mean[:], in1=mean[:],
                                    op=mybir.AluOpType.mult)
            nc.vector.tensor_tensor(out=var[:], in0=ex2[:], in1=var[:],
                                    op=mybir.AluOpType.subtract)
            rstd = small.tile([C, 1], F32)
            nc.scalar.activation(out=rstd[:], in_=var[:],
                                 func=mybir.ActivationFunctionType.Sqrt,
                                 bias=epst[:])
            nc.vector.reciprocal(out=rstd[:], in_=rstd[:])
            scl = small.tile([C, 1], F32)
            nc.vector.tensor_tensor(out=scl[:], in0=gam[:], in1=rstd[:],
                                    op=mybir.AluOpType.mult)
            sht = small.tile([C, 1], F32)
            nc.vector.tensor_tensor(out=sht[:], in0=mean[:], in1=scl[:],
                                    op=mybir.AluOpType.mult)
            nc.vector.tensor_tensor(out=sht[:], in0=bet[:], in1=sht[:],
                                    op=mybir.AluOpType.subtract)

            # ---------- normalize + ReLU (row-major) + store ----------
            # broadcast scale/shift across partitions: diag(v) built on ACT
            # (identity x per-partition scale), then ones^T @ diag replicates
            # the row vector to all 128 partitions
            diag_t = small.tile([C, C], F32, tag="diag")
            ones_t = singles.tile([C, 128], F32)
            nc.vector.memset(ones_t[:], 1.0)
            scb = singles.tile([128, AT * C], F32, tag="scb")
            shb = singles.tile([128, AT * C], F32, tag="shb")
            for vec, wide in ((scl, scb), (sht, shb)):
                nc.scalar.activation(
                    out=diag_t[:], in_=identf[:],
                    func=mybir.ActivationFunctionType.Copy, scale=vec[:],
                )
                pb = pout.tile([128, C], F32, tag="bcast")
                nc.tensor.matmul(out=pb[:], lhsT=ones_t[:], rhs=diag_t[:],
                                 start=True, stop=True)
                for a in range(AT):
                    nc.vector.tensor_copy(
                        out=wide[:, a * C:(a + 1) * C], in_=pb[:])
            for t in range(cfg.nt):
                yb = outsb.tile([128, AT * C], F32, tag="y")
                nc.vector.tensor_tensor(
                    out=yb[:], in0=conv_rm[:, t * AT * C:(t + 1) * AT * C],
                    in1=scb[:], op=mybir.AluOpType.mult,
                )
                nc.vector.tensor_tensor(
                    out=yb[:], in0=yb[:], in1=shb[:], op=mybir.AluOpType.add,
                )
                ob = outsb.tile([128, AT * C], F32)
                nc.scalar.activation(
                    out=ob[:], in_=yb[:],
                    func=mybir.ActivationFunctionType.Relu,
                )
                nc.sync.dma_start(
                    out=outp[t * TR:(t + 1) * TR, :].rearrange(
                        "(s p) c -> p s c", p=128
                    ),
                    in_=ob[:].rearrange("p (s c) -> p s c", c=C),
                )

    nc.compile()
    return nc


def make_in_maps(cfg: Cfg, feats, W, gamma, beta, nbr_idx, mask):
    feats_p = np.concatenate(
        [np.asarray(feats, np.float32),
         np.zeros((1, cfg.c), np.float32)], axis=0
    ).astype(BF16_NP)
    wflat = np.ascontiguousarray(
        np.asarray(W, np.float32).reshape(cfg.k * cfg.c, cfg.c)
    ).astype(BF16_NP)
    gam = np.ascontiguousarray(np.asarray(gamma, np.float32).reshape(cfg.c, 1))
    bet = np.ascontiguousarray(np.asarray(beta, np.float32).reshape(cfg.c, 1))
    nbr_idx = np.asarray(nbr_idx, np.int32)
    mask = np.asarray(mask, np.int32)
    pad = cfg.shard_pad - cfg.shard
    in_maps = []
    for core in range(cfg.n_cores):
        sl = slice(core * cfg.shard, (core + 1) * cfg.shard)
        idx_s = np.concatenate(
            [nbr_idx[:, sl], np.zeros((cfg.k, pad), np.int32)], axis=1)
        msk_s = np.concatenate(
            [mask[:, sl], np.zeros((cfg.k, pad), np.int32)], axis=1)
        # [k, nsub, 128] -> [128, nsub, k]
        idxT = np.ascontiguousarray(
            idx_s.reshape(cfg.k, cfg.nsub, 128).transpose(2, 1, 0)
        ).reshape(128, cfg.nsub * cfg.k)
        mskT = np.ascontiguousarray(
            msk_s.reshape(cfg.k, cfg.nsub, 128).transpose(2, 1, 0)
        ).reshape(128, cfg.nsub * cfg.k)
        centr = np.concatenate(
            [feats_p[core * cfg.shard:(core + 1) * cfg.shard],
             np.zeros((pad, cfg.c), BF16_NP)], axis=0)
        in_maps.append({
            "feats": feats_p, "wflat": wflat, "gamma": gam, "beta": bet,
            "idxT": idxT, "maskT": mskT, "center": centr,
        })
    return in_maps


_CACHE = {}


def _get_nc(cfg: Cfg):
    key = (cfg.n, cfg.c, cfg.k, cfg.n_cores, cfg.tile_rows, cfg.n_queues,
           cfg.act_copies, cfg.zero_row)
    if key not in _CACHE:
        _CACHE[key] = build_kernel(cfg)
    return _CACHE[key]


def run_hw(cfg: Cfg, inputs, trace=False):
    nc = _get_nc(cfg)
    in_maps = make_in_maps(cfg, **inputs)
    res = run_bass_kernel_spmd(
        nc, in_maps, core_ids=list(range(cfg.n_cores)), trace=trace
    )
    out = np.concatenate(
        [res.results[c]["out"][: cfg.shard] for c in range(cfg.n_cores)], axis=0
    )
    return np.ascontiguousarray(out, dtype=np.float32), res


def kernel(feats, W, gamma, beta, nbr_idx, mask):
    cfg = Cfg(n=feats.shape[0], c=feats.shape[1], k=W.shape[0])
    out, _ = run_hw(cfg, dict(feats=feats, W=W, gamma=gamma, beta=beta,
                              nbr_idx=nbr_idx, mask=mask))
    return out


# revision 38
# speedup vs baseline: 1.0051x; 1.0051x over previous
"""Trainium2 Bass kernel for nn_BasicConvolutionBlock (gather-GEMM sparse conv + BN + ReLU).

Math (see reference): for each of K=27 kernel offsets,
    conv += (feats[nbr_idx[k]] * mask[k,:,None]) @ W[k]
then train-mode BatchNorm over the N axis (global mean/var per channel) + ReLU.

Distribution: voxel dim N sharded over 8 cores (data parallel). feats table and
weights replicated to every core; each core gathers its shard's neighbors
locally via indirect DMA. BatchNorm stats are all-reduced across cores.

Perf notes (from HW traces): the bottleneck is the Pool/SWDGE engine --
each indirect gather instruction carries exactly ONE row offset per
partition (128 descriptors; multi-offset encodings are unimplemented in
the DGE: indirection is pinned to the source's slowest dim and walrus
only lowers the 2-dim dest form -- HW-probed), and costs ~1.16us of
serialized descriptor-generation time. 196 subtiles x 26 non-center
offsets = 5096 instructions ~= 5.9ms Pool time; everything else hides
under it. Optimizations vs the first working version (9.0ms -> ~6ms):
  - bf16 pipeline (host-converted feats/weights): halves gather bytes,
    G memset cost (DVE 2x), PE transpose cycles (1 vs 2 cyc/row)
  - center k-plane (identity map, contiguous) fetched via HWDGE (sync
    engine), off the Pool engine; collective staging DMAs also on sync
  - masked taps OOB-skipped via bounds reg (cheaper in SWDGE gen + SDMA
    than gathering a zero row: dummies beat real descriptors, measured)
  - G buffers pre-zeroed 2 tiles AHEAD so the gather stream never waits
    on the DVE FIFO (memset would otherwise queue behind PSUM->SBUF
    copies that wait on PE transposes, coupling Pool to PE each tile)
  - PSUM->SBUF rhs copies alternate DVE/ACT (both 1 PSUM input max)
Accumulation stays f32 in PSUM; BN stats are computed in f32.

Per-core pipeline:
  1. masked index fold: idx' = mask ? idx : N+1  (> bounds -> skipped, G
     pre-zeroed so masked taps contribute 0)
  2. per 512-row tile: 4x26 per-(subtile,k) indirect gathers into
     [128p, 4a, 27k, 64c] bf16 (per-tile queue blocks); per k-pair:
     PE-transpose subtiles to [128kc, 512pt] PSUM, DVE/ACT copy to SBUF
     bf16, PE matmul accumulating into PSUM [64, 512]
  3. per-tile partial stats (sum / sumsq on DVE); conv kept in SBUF bf16
  4. AllReduce [64,2] stats -> scale/shift; ACT fused affine+ReLU;
     PE transpose back to row-major; DMA out.
"""

import os
import sys

sys.path.insert(0, "/opt/trn_rl_repo")

import numpy as np
import ml_dtypes


def _install_ntff_hook_module():
    """Provide antenv.axon_hooks (NTFF profiling under axon) if the image
    lacks it, so run_bass_kernel_spmd(trace=True) can report exec_time_ns."""
    import importlib
    try:
        importlib.import_module("antenv.axon_hooks")
        return
    except ImportError:
        pass
    import contextlib
    import ctypes
    import types

    so_path = "/opt/axon/libaxon_pjrt.so"
    mod = types.ModuleType("antenv.axon_hooks")
    state = {"hook": None, "tried": False}

    def set_axon_ntff_profile_hook(hook):
        state["hook"] = hook

    def _build_hook():
        if not os.path.exists(so_path):
            return None
        lib = ctypes.CDLL(so_path)
        if not hasattr(lib, "axon_start_nrt_profile"):
            return None
        lib.axon_start_nrt_profile.argtypes = [
            ctypes.POINTER(ctypes.c_int64), ctypes.c_size_t]
        lib.axon_start_nrt_profile.restype = ctypes.c_int64
        lib.axon_stop_nrt_profile.argtypes = [ctypes.c_char_p]
        lib.axon_stop_nrt_profile.restype = ctypes.c_int64

        @contextlib.contextmanager
        def _hook(output_dir, device_ids):
            import jax
            jax.devices()
            if device_ids:
                ids = (ctypes.c_int64 * len(device_ids))(*device_ids)
                rc = lib.axon_start_nrt_profile(ids, len(device_ids))
            else:
                rc = lib.axon_start_nrt_profile(None, 0)
            if rc != 0:
                raise RuntimeError(f"axon_start_nrt_profile rc={rc}")
            try:
                yield
            finally:
                n = lib.axon_stop_nrt_profile(str(output_dir).encode())
                print(f"ntff profile: {n} file(s) -> {output_dir}",
                      file=sys.stderr)

        return _hook

    def get_axon_ntff_profile_hook():
        if state["hook"] is None and not state["tried"]:
            state["tried"] = True
            state["hook"] = _build_hook()
        return state["hook"]

    mod.set_axon_ntff_profile_hook = set_axon_ntff_profile_hook
    mod.get_axon_ntff_profile_hook = get_axon_ntff_profile_hook
    sys.modules["antenv.axon_hooks"] = mod


_install_ntff_hook_module()

import concourse.bass as bass
import concourse.bacc as bacc
import concourse.tile as tile
from concourse import mybir
from concourse.bass_utils import run_bass_kernel_spmd
from concourse.masks import make_identity

F32 = mybir.dt.float32
BF16 = mybir.dt.bfloat16
I32 = mybir.dt.int32
BF16_NP = np.dtype(ml_dtypes.bfloat16)


def _indirect_gather_q(nc, out_ap, in_ap, offset_ap, queue: str,
                       bounds_reg=None):
    """bass.indirect_dma_start (gather form) with a selectable SWDGE queue and
    an optional pre-made bounds register (indices > bound are skipped).

    The offset AP may carry many offsets per partition. The DGE pins the
    indirection dimension to the SOURCE tensor's slowest dim and fetches a
    new index each time the lockstep iteration crosses it, so the AP
    contract for an M-offsets-per-partition gather is:
      src:    [M, elem]   (outer num_elem = per-channel index count; its
                           stride is discarded -- the fetched index * coef
                           addresses the row)
      dest:   [128, M, elem]  (NOT flattened: a merged free dim becomes one
                           index + M*elem contiguous bytes)
      offset: [128, M] int32 (4B index walk per channel)
    This amortizes the ~1us SWDGE fixed cost over M*128 descriptors."""
    gp = nc.gpsimd
    table_rows = in_ap.shape[0]
    if len(out_ap.shape) > 2:
        m_per_ch = 1
        for s in out_ap.shape[1:-1]:
            m_per_ch *= s
        out_ap = out_ap.rearrange(
            f"p {' '.join(chr(97 + i) for i in range(len(out_ap.shape) - 2))} c"
            f" -> p ({' '.join(chr(97 + i) for i in range(len(out_ap.shape) - 2))}) c"
        )
        assert out_ap.shape == (128, m_per_ch, in_ap.shape[-1])
    offset_ap = offset_ap.opt()
    out_l = gp.lower_ap_dma(out_ap, for_indirect_dma=True)
    in_l = gp.lower_ap_dma(in_ap, for_indirect_dma=True)
    assert len(in_l) == 1 and len(out_l) == 1
    off_l = gp.lower_ap_dma(offset_ap)
    assert len(off_l) == 1
    in_l.append(off_l[0])
    ap_shape = in_ap.shape
    coef = 1
    for i in range(1, len(ap_shape)):
        coef *= ap_shape[i]
    in_l[0].dynamic_ap_info = mybir.DynamicAccessPatternInfo(
        c=0,
        actual_ap=out_ap.ap,
        indirect_dim_max_index=table_rows,
        offset_expr=[
            mybir.DynamicAccessPatternOffsetExpr(
                coef=coef,
                aff_expr=mybir.DynamicAccessPatternOffsetExprAffExpr(
                    kind="IndirectArgId", arg_id=1,
                ),
            )
        ],
    )
    if bounds_reg is not None:
        in_l = in_l + [gp.lower_val_access(bounds_reg)]
    return gp.add_instruction(
        mybir.InstDMACopy(
            name=nc.get_next_instruction_name(),
            queue=queue,
            mode="Copy",
            ins=in_l,
            outs=out_l,
            oob_is_err=False,
            cce_op=mybir.AluOpType.bypass,
        )
    )


class Cfg:
    def __init__(self, n=200000, c=64, k=27, n_cores=8, tile_rows=512,
                 eps=1e-5, n_queues=4, act_copies=True, zero_row=False):
        # zero_row: masked taps read the table's zero row (row n) instead of
        # being OOB-skipped -- drops the bounds check and the G memset at the
        # cost of extra (DRAM-row-hot) gather traffic.
        self.zero_row = zero_row
        assert n % n_cores == 0
        self.n, self.c, self.k, self.n_cores = n, c, k, n_cores
        self.eps = eps
        self.shard = n // n_cores
        self.nsub = (self.shard + 127) // 128          # 128-row subtiles
        self.shard_pad = self.nsub * 128
        self.tile_rows = tile_rows                     # rows per PSUM tile
        self.a_per_tile = tile_rows // 128             # subtiles per tile
        assert self.nsub % self.a_per_tile == 0
        self.nt = self.shard_pad // tile_rows          # tiles per core
        self.npair = (k + 1) // 2                      # last pair may be single
        self.n_queues = n_queues
        self.act_copies = act_copies                   # PSUM->SBUF copies split DVE/ACT
        self.table_rows = n + 1                        # + zero row (unused; bounds pad)


def build_kernel(cfg: Cfg):
    nc = bacc.Bacc("TRN2", target_bir_lowering=False, debug=False,
                   num_devices=cfg.n_cores, num_swdge_queues=cfg.n_queues)
    C, K = cfg.c, cfg.k
    TR, AT = cfg.tile_rows, cfg.a_per_tile
    KP = K

    feats = nc.dram_tensor("feats", [cfg.table_rows, C], BF16, kind="ExternalInput")
    wflat = nc.dram_tensor("wflat", [K * C, C], BF16, kind="ExternalInput")
    gamma = nc.dram_tensor("gamma", [C, 1], F32, kind="ExternalInput")
    beta = nc.dram_tensor("beta", [C, 1], F32, kind="ExternalInput")
    # host-transposed indices/mask: [128, nsub, k] with (p, a, k) = idx[k, a*128+p]
    idxT = nc.dram_tensor("idxT", [128, cfg.nsub * K], I32, kind="ExternalInput")
    maskT = nc.dram_tensor("maskT", [128, cfg.nsub * K], I32, kind="ExternalInput")
    # center k-plane of the gather is the identity map over the core's own
    # shard -- a contiguous read that HWDGE (sync engine) can do, keeping
    # those 196 transfers off the serialized Pool/SWDGE engine.
    center = nc.dram_tensor("center", [cfg.shard_pad, C], BF16, kind="ExternalInput")
    outp = nc.dram_tensor("out", [cfg.shard_pad, C], F32, kind="ExternalOutput")

    with tile.TileContext(nc) as tc:
        with (
            tc.tile_pool(name="singles", bufs=1) as singles,
            tc.tile_pool(name="gpool", bufs=1) as gpool,
            tc.tile_pool(name="trp", bufs=3, space="PSUM") as trp,
            tc.tile_pool(name="rhsp", bufs=4) as rhsp,
            tc.tile_pool(name="pacc", bufs=2, space="PSUM") as pacc,
            tc.tile_pool(name="pout", bufs=2, space="PSUM") as pout,
            tc.tile_pool(name="outsb", bufs=3) as outsb,
            tc.tile_pool(name="small", bufs=4) as small,
            tc.tile_pool(name="dram", bufs=1, space="DRAM") as dram,
        ):
            # ---------- constants ----------
            ident = singles.tile([128, 128], BF16)
            make_identity(nc, ident[:])
            identf = singles.tile([C, C], F32)
            make_identity(nc, identf[:])

            w_sb = singles.tile([128, cfg.npair * C], BF16)
            npair_full = K // 2  # pairs with both k's real
            nc.vector.memset(w_sb[:], 0.0)
            nc.sync.dma_start(
                out=w_sb[:, : npair_full * C].rearrange("p (j c) -> p j c", j=npair_full),
                in_=wflat[: npair_full * 128, :].rearrange("(j p) c -> p j c", p=128),
            )
            if K % 2:
                # trailing single k in the top 64 partitions of the last slot
                nc.sync.dma_start(
                    out=w_sb[:C, npair_full * C:(npair_full + 1) * C],
                    in_=wflat[(K - 1) * C: K * C, :],
                )

            gam = singles.tile([C, 1], F32)
            bet = singles.tile([C, 1], F32)
            nc.sync.dma_start(out=gam[:], in_=gamma[:])
            nc.sync.dma_start(out=bet[:], in_=beta[:])
            epst = singles.tile([C, 1], F32)
            nc.vector.memset(epst[:], cfg.eps)

            # ---------- masked index fold ----------
            # idx' = mask ? idx : (n+1 | n). With bounds check (bound = n)
            # n+1 entries are skipped by the DGE and the pre-zeroed G supplies
            # the zeros; in zero_row mode they read the zero row n directly.
            idx_sb = singles.tile([128, cfg.nsub, KP], I32)
            nc.vector.memset(idx_sb[:], cfg.n if cfg.zero_row else cfg.n + 1)
            with tc.tile_pool(name="idxstage", bufs=1) as stage:
                idx_raw = stage.tile([128, cfg.nsub, KP], I32)
                msk_raw = stage.tile([128, cfg.nsub, KP], I32)
                nc.sync.dma_start(out=idx_raw[:],
                                  in_=idxT[:].rearrange("p (a k) -> p a k", k=K))
                nc.sync.dma_start(out=msk_raw[:],
                                  in_=maskT[:].rearrange("p (a k) -> p a k", k=K))
                nc.vector.copy_predicated(
                    out=idx_sb[:], mask=msk_raw[:], data=idx_raw[:]
                )

            conv_sb = singles.tile([C, cfg.shard_pad], BF16)
            stats_s = singles.tile([C, cfg.nt], F32)
            stats_q = singles.tile([C, cfg.nt], F32)

            # ---------- main conv loop ----------
            # HW constraint (probed): one indirect DMA consumes exactly one
            # row offset per partition -- 128 descriptors per instruction.
            bc_reg = None if cfg.zero_row else nc.gpsimd.to_reg(cfg.n)
            center_k = K // 2
            # Pre-zero G buffers AHEAD of use: the memset for tile t+lead is
            # emitted before tile t's PSUM->SBUF copies so it never queues
            # behind them in the DVE FIFO (which would couple the Pool gather
            # stream to PE transpose latency every tile).
            lead = 2
            gq = []
            for i in range(min(lead, cfg.nt)):
                Gn = gpool.tile([128, AT, KP, C], BF16, tag=f"g{i}")
                if not cfg.zero_row:
                    nc.vector.memset(Gn[:], 0.0)
                gq.append(Gn)
            for t in range(cfg.nt):
                G = gq.pop(0)
                if t + lead < cfg.nt:
                    Gn = gpool.tile([128, AT, KP, C], BF16, tag=f"g{(t + lead) % (lead + 2)}")
                    if not cfg.zero_row:
                        nc.vector.memset(Gn[:], 0.0)
                    gq.append(Gn)
                nc.sync.dma_start(
                    out=G[:, :, center_k, :],
                    in_=center[t * TR:(t + 1) * TR, :].rearrange(
                        "(s p) c -> p s c", p=128),
                )
                for s in range(AT):
                    a = t * AT + s
                    for k in range(KP):
                        if k == center_k:
                            continue
                        # one queue per tile (blocked): consecutive Pool
                        # instructions share the SWDGE ring context; drain
                        # still overlaps across tiles on rotating queues
                        q = t % cfg.n_queues
                        _indirect_gather_q(
                            nc,
                            out_ap=G[:, s, k, :],
                            in_ap=feats[:],
                            offset_ap=idx_sb[:, a, k:k + 1],
                            queue=f"qPoolDynamic{q or ''}",
                            bounds_reg=bc_reg,
                        )

                acc = pacc.tile([C, TR], F32)
                for j in range(cfg.npair):
                    single = (j == cfg.npair - 1) and (K % 2 == 1)
                    np_ = C if single else 2 * C
                    ptr = trp.tile([128, TR], BF16)
                    for s in range(AT):
                        nc.tensor.transpose(
                            out=ptr[:np_, s * 128:(s + 1) * 128],
                            in_=G[:, s, 2 * j:2 * j + (1 if single else 2), :],
                            identity=ident[:],
                        )
                    rhs = rhsp.tile([128, TR], BF16)
                    if cfg.act_copies and (j % 2 == 1):
                        nc.scalar.activation(
                            out=rhs[:np_, :], in_=ptr[:np_, :],
                            func=mybir.ActivationFunctionType.Copy,
                        )
                    else:
                        nc.vector.tensor_copy(out=rhs[:np_, :], in_=ptr[:np_, :])
                    nc.tensor.matmul(
                        out=acc[:],
                        lhsT=w_sb[:np_, j * C:(j + 1) * C],
                        rhs=rhs[:np_, :],
                        start=(j == 0),
                        stop=(j == cfg.npair - 1),
                    )

                # partial BN stats (all on DVE: keep ACT's function table on
                # Copy all run) + conv store
                nc.vector.reduce_sum(
                    out=stats_s[:, t:t + 1], in_=acc[:], axis=mybir.AxisListType.X
                )
                cs = conv_sb[:, t * TR:(t + 1) * TR]
                nc.vector.tensor_copy(out=cs, in_=acc[:])
                # sumsq from the SBUF bf16 copy (walrus allows only one PSUM
                # input per DVE op); consistent with the normalized values
                sq = small.tile([C, TR], F32)
                nc.vector.tensor_tensor(out=sq[:], in0=cs, in1=cs,
                                        op=mybir.AluOpType.mult)
                nc.vector.reduce_sum(
                    out=stats_q[:, t:t + 1], in_=sq[:], axis=mybir.AxisListType.X
                )

            # ---------- global BN stats (AllReduce) ----------
            sums = small.tile([C, 2], F32)
            nc.vector.reduce_sum(out=sums[:, 0:1], in_=stats_s[:], axis=mybir.AxisListType.X)
            nc.vector.reduce_sum(out=sums[:, 1:2], in_=stats_q[:], axis=mybir.AxisListType.X)
            cc_in = dram.tile([C, 2], F32)
            cc_out = dram.tile([C, 2], F32)
            nc.sync.dma_start(out=cc_in[:], in_=sums[:])
            nc.gpsimd.collective_compute(
                "AllReduce",
                mybir.AluOpType.add,
                replica_groups=[list(range(cfg.n_cores))],
                ins=[cc_in.opt()],
                outs=[cc_out.opt()],
            )
            gsum = small.tile([C, 2], F32)
            nc.sync.dma_start(out=gsum[:], in_=cc_out[:])

            mean = small.tile([C, 1], F32)
            ex2 = small.tile([C, 1], F32)
            nc.scalar.mul(out=mean[:], in_=gsum[:, 0:1], mul=1.0 / cfg.n)
            nc.scalar.mul(out=ex2[:], in_=gsum[:, 1:2], mul=1.0 / cfg.n)
            var = small.tile([C, 1], F32)
            nc.vector.tensor_tensor(out=var[:], in0=mean[:], in1=mean[:],
                                    op=mybir.AluOpType.mult)
            nc.vector.tensor_tensor(out=var[:], in0=ex2[:], in1=var[:],
                                    op=mybir.AluOpType.subtract)
            rstd = small.tile([C, 1], F32)
            nc.scalar.activation(out=rstd[:], in_=var[:],
                                 func=mybir.ActivationFunctionType.Sqrt,
                                 bias=epst[:])
            nc.vector.reciprocal(out=rstd[:], in_=rstd[:])
            scl = small.tile([C, 1], F32)
            nc.vector.tensor_tensor(out=scl[:], in0=gam[:], in1=rstd[:],
                                    op=mybir.AluOpType.mult)
            sht = small.tile([C, 1], F32)
            nc.vector.tensor_tensor(out=sht[:], in0=mean[:], in1=scl[:],
                                    op=mybir.AluOpType.mult)
            nc.vector.tensor_tensor(out=sht[:], in0=bet[:], in1=sht[:],
                                    op=mybir.AluOpType.subtract)

            # ---------- normalize + ReLU + transpose back + store ----------
            for t in range(cfg.nt):
                nb = rhsp.tile([C, TR], F32, tag="norm")
                nc.scalar.activation(
                    out=nb[:], in_=conv_sb[:, t * TR:(t + 1) * TR],
                    func=mybir.ActivationFunctionType.Relu,
                    bias=sht[:], scale=scl[:],
                )
                po = pout.tile([128, AT * C], F32)
                for s in range(AT):
                    nc.tensor.transpose(
                        out=po[:, s * C:(s + 1) * C],
                        in_=nb[:, s * 128:(s + 1) * 128],
                        identity=identf[:],
                    )
                ob = outsb.tile([128, AT * C], F32)
                nc.vector.tensor_copy(out=ob[:], in_=po[:])
                nc.sync.dma_start(
                    out=outp[t * TR:(t + 1) * TR, :].rearrange(
                        "(s p) c -> p s c", p=128
                    ),
                    in_=ob[:].rearrange("p (s c) -> p s c", c=C),
                )

    nc.compile()
    return nc


def make_in_maps(cfg: Cfg, feats, W, gamma, beta, nbr_idx, mask):
    feats_p = np.concatenate(
        [np.asarray(feats, np.float32),
         np.zeros((1, cfg.c), np.float32)], axis=0
    ).astype(BF16_NP)
    wflat = np.ascontiguousarray(
        np.asarray(W, np.float32).reshape(cfg.k * cfg.c, cfg.c)
    ).astype(BF16_NP)
    gam = np.ascontiguousarray(np.asarray(gamma, np.float32).reshape(cfg.c, 1))
    bet = np.ascontiguousarray(np.asarray(beta, np.float32).reshape(cfg.c, 1))
    nbr_idx = np.asarray(nbr_idx, np.int32)
    mask = np.asarray(mask, np.int32)
    pad = cfg.shard_pad - cfg.shard
    in_maps = []
    for core in range(cfg.n_cores):
        sl = slice(core * cfg.shard, (core + 1) * cfg.shard)
        idx_s = np.concatenate(
            [nbr_idx[:, sl], np.zeros((cfg.k, pad), np.int32)], axis=1)
        msk_s = np.concatenate(
            [mask[:, sl], np.zeros((cfg.k, pad), np.int32)], axis=1)
        # [k, nsub, 128] -> [128, nsub, k]
        idxT = np.ascontiguousarray(
            idx_s.reshape(cfg.k, cfg.nsub, 128).transpose(2, 1, 0)
        ).reshape(128, cfg.nsub * cfg.k)
        mskT = np.ascontiguousarray(
            msk_s.reshape(cfg.k, cfg.nsub, 128).transpose(2, 1, 0)
        ).reshape(128, cfg.nsub * cfg.k)
        centr = np.concatenate(
            [feats_p[core * cfg.shard:(core + 1) * cfg.shard],
             np.zeros((pad, cfg.c), BF16_NP)], axis=0)
        in_maps.append({
            "feats": feats_p, "wflat": wflat, "gamma": gam, "beta": bet,
            "idxT": idxT, "maskT": mskT, "center": centr,
        })
    return in_maps


_CACHE = {}


def _get_nc(cfg: Cfg):
    key = (cfg.n, cfg.c, cfg.k, cfg.n_cores, cfg.tile_rows, cfg.n_queues,
           cfg.act_copies, cfg.zero_row)
    if key not in _CACHE:
        _CACHE[key] = build_kernel(cfg)
    return _CACHE[key]


def run_hw(cfg: Cfg, inputs, trace=False):
    nc = _get_nc(cfg)
    in_maps = make_in_maps(cfg, **inputs)
    res = run_bass_kernel_spmd(
        nc, in_maps, core_ids=list(range(cfg.n_cores)), trace=trace
    )
    out = np.concatenate(
        [res.results[c]["out"][: cfg.shard] for c in range(cfg.n_cores)], axis=0
    )
    return np.ascontiguousarray(out, dtype=np.float32), res


def kernel(feats, W, gamma, beta, nbr_idx, mask):
    cfg = Cfg(n=feats.shape[0], c=feats.shape[1], k=W.shape[0])
    out, _ = run_hw(cfg, dict(feats=feats, W=W, gamma=gamma, beta=beta,
                              nbr_idx=nbr_idx, mask=mask))
    return out


# revision 39
# speedup vs baseline: 1.0078x; 1.0027x over previous
"""Trainium2 Bass kernel for nn_BasicConvolutionBlock (gather-GEMM sparse conv + BN + ReLU).

Math (see reference): for each of K=27 kernel offsets,
    conv += (feats[nbr_idx[k]] * mask[k,:,None]) @ W[k]
then train-mode BatchNorm over the N axis (global mean/var per channel) + ReLU.

Distribution: voxel dim N sharded over 8 cores (data parallel). feats table and
weights replicated to every core; each core gathers its shard's neighbors
locally via indirect DMA. BatchNorm stats are all-reduced across cores.

Perf notes (from HW traces): the bottleneck is the Pool/SWDGE engine --
each indirect gather instruction carries exactly ONE row offset per
partition (128 descriptors; multi-offset encodings are unimplemented in
the DGE: indirection is pinned to the source's slowest dim and walrus
only lowers the 2-dim dest form -- HW-probed), and costs ~1.16us of
serialized descriptor-generation time. 196 subtiles x 26 non-center
offsets = 5096 instructions ~= 5.9ms Pool time; everything else hides
under it. Optimizations vs the first working version (9.0ms -> ~6ms):
  - bf16 pipeline (host-converted feats/weights): halves gather bytes,
    G memset cost (DVE 2x), PE transpose cycles (1 vs 2 cyc/row)
  - center k-plane (identity map, contiguous) fetched via HWDGE (sync
    engine), off the Pool engine; collective staging DMAs also on sync
  - masked taps OOB-skipped via bounds reg (cheaper in SWDGE gen + SDMA
    than gathering a zero row: dummies beat real descriptors, measured)
  - G buffers pre-zeroed 2 tiles AHEAD so the gather stream never waits
    on the DVE FIFO (memset would otherwise queue behind PSUM->SBUF
    copies that wait on PE transposes, coupling Pool to PE each tile)
  - PSUM->SBUF rhs copies alternate DVE/ACT (both 1 PSUM input max)
Accumulation stays f32 in PSUM; BN stats are computed in f32.

Per-core pipeline:
  1. masked index fold: idx' = mask ? idx : N+1  (> bounds -> skipped, G
     pre-zeroed so masked taps contribute 0)
  2. per 512-row tile: 4x26 per-(subtile,k) indirect gathers into
     [128p, 4a, 27k, 64c] bf16 (per-tile queue blocks); per k-pair:
     PE-transpose subtiles to [128kc, 512pt] PSUM, DVE/ACT copy to SBUF
     bf16, PE matmul accumulating into PSUM [64, 512]
  3. per-tile partial stats (sum / sumsq on DVE); conv kept in SBUF bf16
  4. AllReduce [64,2] stats -> scale/shift; ACT fused affine+ReLU;
     PE transpose back to row-major; DMA out.
"""

import os
import sys

sys.path.insert(0, "/opt/trn_rl_repo")

import numpy as np
import ml_dtypes


def _install_ntff_hook_module():
    """Provide antenv.axon_hooks (NTFF profiling under axon) if the image
    lacks it, so run_bass_kernel_spmd(trace=True) can report exec_time_ns."""
    import importlib
    try:
        importlib.import_module("antenv.axon_hooks")
        return
    except ImportError:
        pass
    import contextlib
    import ctypes
    import types

    so_path = "/opt/axon/libaxon_pjrt.so"
    mod = types.ModuleType("antenv.axon_hooks")
    state = {"hook": None, "tried": False}

    def set_axon_ntff_profile_hook(hook):
        state["hook"] = hook

    def _build_hook():
        if not os.path.exists(so_path):
            return None
        lib = ctypes.CDLL(so_path)
        if not hasattr(lib, "axon_start_nrt_profile"):
            return None
        lib.axon_start_nrt_profile.argtypes = [
            ctypes.POINTER(ctypes.c_int64), ctypes.c_size_t]
        lib.axon_start_nrt_profile.restype = ctypes.c_int64
        lib.axon_stop_nrt_profile.argtypes = [ctypes.c_char_p]
        lib.axon_stop_nrt_profile.restype = ctypes.c_int64

        @contextlib.contextmanager
        def _hook(output_dir, device_ids):
            import jax
            jax.devices()
            if device_ids:
                ids = (ctypes.c_int64 * len(device_ids))(*device_ids)
                rc = lib.axon_start_nrt_profile(ids, len(device_ids))
            else:
                rc = lib.axon_start_nrt_profile(None, 0)
            if rc != 0:
                raise RuntimeError(f"axon_start_nrt_profile rc={rc}")
            try:
                yield
            finally:
                n = lib.axon_stop_nrt_profile(str(output_dir).encode())
                print(f"ntff profile: {n} file(s) -> {output_dir}",
                      file=sys.stderr)

        return _hook

    def get_axon_ntff_profile_hook():
        if state["hook"] is None and not state["tried"]:
            state["tried"] = True
            state["hook"] = _build_hook()
        return state["hook"]

    mod.set_axon_ntff_profile_hook = set_axon_ntff_profile_hook
    mod.get_axon_ntff_profile_hook = get_axon_ntff_profile_hook
    sys.modules["antenv.axon_hooks"] = mod


_install_ntff_hook_module()

import concourse.bass as bass
import concourse.bacc as bacc
import concourse.tile as tile
from concourse import mybir
from concourse.bass_utils import run_bass_kernel_spmd
from concourse.masks import make_identity

F32 = mybir.dt.float32
BF16 = mybir.dt.bfloat16
I32 = mybir.dt.int32
BF16_NP = np.dtype(ml_dtypes.bfloat16)


def _indirect_gather_q(nc, out_ap, in_ap, offset_ap, queue: str,
                       bounds_reg=None):
    """bass.indirect_dma_start (gather form) with a selectable SWDGE queue and
    an optional pre-made bounds register (indices > bound are skipped).

    The offset AP may carry many offsets per partition. The DGE pins the
    indirection dimension to the SOURCE tensor's slowest dim and fetches a
    new index each time the lockstep iteration crosses it, so the AP
    contract for an M-offsets-per-partition gather is:
      src:    [M, elem]   (outer num_elem = per-channel index count; its
                           stride is discarded -- the fetched index * coef
                           addresses the row)
      dest:   [128, M, elem]  (NOT flattened: a merged free dim becomes one
                           index + M*elem contiguous bytes)
      offset: [128, M] int32 (4B index walk per channel)
    This amortizes the ~1us SWDGE fixed cost over M*128 descriptors."""
    gp = nc.gpsimd
    table_rows = in_ap.shape[0]
    if len(out_ap.shape) > 2:
        m_per_ch = 1
        for s in out_ap.shape[1:-1]:
            m_per_ch *= s
        out_ap = out_ap.rearrange(
            f"p {' '.join(chr(97 + i) for i in range(len(out_ap.shape) - 2))} c"
            f" -> p ({' '.join(chr(97 + i) for i in range(len(out_ap.shape) - 2))}) c"
        )
        assert out_ap.shape == (128, m_per_ch, in_ap.shape[-1])
    offset_ap = offset_ap.opt()
    out_l = gp.lower_ap_dma(out_ap, for_indirect_dma=True)
    in_l = gp.lower_ap_dma(in_ap, for_indirect_dma=True)
    assert len(in_l) == 1 and len(out_l) == 1
    off_l = gp.lower_ap_dma(offset_ap)
    assert len(off_l) == 1
    in_l.append(off_l[0])
    ap_shape = in_ap.shape
    coef = 1
    for i in range(1, len(ap_shape)):
        coef *= ap_shape[i]
    in_l[0].dynamic_ap_info = mybir.DynamicAccessPatternInfo(
        c=0,
        actual_ap=out_ap.ap,
        indirect_dim_max_index=table_rows,
        offset_expr=[
            mybir.DynamicAccessPatternOffsetExpr(
                coef=coef,
                aff_expr=mybir.DynamicAccessPatternOffsetExprAffExpr(
                    kind="IndirectArgId", arg_id=1,
                ),
            )
        ],
    )
    if bounds_reg is not None:
        in_l = in_l + [gp.lower_val_access(bounds_reg)]
    return gp.add_instruction(
        mybir.InstDMACopy(
            name=nc.get_next_instruction_name(),
            queue=queue,
            mode="Copy",
            ins=in_l,
            outs=out_l,
            oob_is_err=False,
            cce_op=mybir.AluOpType.bypass,
        )
    )


class Cfg:
    def __init__(self, n=200000, c=64, k=27, n_cores=8, tile_rows=512,
                 eps=1e-5, n_queues=4, act_copies=True, zero_row=False):
        # zero_row: masked taps read the table's zero row (row n) instead of
        # being OOB-skipped -- drops the bounds check and the G memset at the
        # cost of extra (DRAM-row-hot) gather traffic.
        self.zero_row = zero_row
        assert n % n_cores == 0
        self.n, self.c, self.k, self.n_cores = n, c, k, n_cores
        self.eps = eps
        self.shard = n // n_cores
        self.nsub = (self.shard + 127) // 128          # 128-row subtiles
        self.shard_pad = self.nsub * 128
        self.tile_rows = tile_rows                     # rows per PSUM tile
        self.a_per_tile = tile_rows // 128             # subtiles per tile
        assert self.nsub % self.a_per_tile == 0
        self.nt = self.shard_pad // tile_rows          # tiles per core
        self.npair = (k + 1) // 2                      # last pair may be single
        self.n_queues = n_queues
        self.act_copies = act_copies                   # PSUM->SBUF copies split DVE/ACT
        self.table_rows = n + 1                        # + zero row (unused; bounds pad)


def build_kernel(cfg: Cfg):
    nc = bacc.Bacc("TRN2", target_bir_lowering=False, debug=False,
                   num_devices=cfg.n_cores, num_swdge_queues=cfg.n_queues)
    C, K = cfg.c, cfg.k
    TR, AT = cfg.tile_rows, cfg.a_per_tile
    KP = K

    feats = nc.dram_tensor("feats", [cfg.table_rows, C], BF16, kind="ExternalInput")
    wflat = nc.dram_tensor("wflat", [K * C, C], BF16, kind="ExternalInput")
    gamma = nc.dram_tensor("gamma", [C, 1], F32, kind="ExternalInput")
    beta = nc.dram_tensor("beta", [C, 1], F32, kind="ExternalInput")
    # host-transposed indices/mask: [128, nsub, k] with (p, a, k) = idx[k, a*128+p]
    idxT = nc.dram_tensor("idxT", [128, cfg.nsub * K], I32, kind="ExternalInput")
    maskT = nc.dram_tensor("maskT", [128, cfg.nsub * K], I32, kind="ExternalInput")
    # center k-plane of the gather is the identity map over the core's own
    # shard -- a contiguous read that HWDGE (sync engine) can do, keeping
    # those 196 transfers off the serialized Pool/SWDGE engine.
    center = nc.dram_tensor("center", [cfg.shard_pad, C], BF16, kind="ExternalInput")
    outp = nc.dram_tensor("out", [cfg.shard_pad, C], F32, kind="ExternalOutput")

    with tile.TileContext(nc) as tc:
        with (
            tc.tile_pool(name="singles", bufs=1) as singles,
            tc.tile_pool(name="gpool", bufs=1) as gpool,
            tc.tile_pool(name="trp", bufs=3, space="PSUM") as trp,
            tc.tile_pool(name="rhsp", bufs=6) as rhsp,
            tc.tile_pool(name="pacc", bufs=2, space="PSUM") as pacc,
            tc.tile_pool(name="pout", bufs=2, space="PSUM") as pout,
            tc.tile_pool(name="outsb", bufs=3) as outsb,
            tc.tile_pool(name="small", bufs=4) as small,
            tc.tile_pool(name="dram", bufs=1, space="DRAM") as dram,
        ):
            # ---------- constants ----------
            ident = singles.tile([128, 128], BF16)
            make_identity(nc, ident[:])
            identf = singles.tile([C, C], F32)
            make_identity(nc, identf[:])

            w_sb = singles.tile([128, cfg.npair * C], BF16)
            npair_full = K // 2  # pairs with both k's real
            nc.vector.memset(w_sb[:], 0.0)
            nc.sync.dma_start(
                out=w_sb[:, : npair_full * C].rearrange("p (j c) -> p j c", j=npair_full),
                in_=wflat[: npair_full * 128, :].rearrange("(j p) c -> p j c", p=128),
            )
            if K % 2:
                # trailing single k in the top 64 partitions of the last slot
                nc.sync.dma_start(
                    out=w_sb[:C, npair_full * C:(npair_full + 1) * C],
                    in_=wflat[(K - 1) * C: K * C, :],
                )

            gam = singles.tile([C, 1], F32)
            bet = singles.tile([C, 1], F32)
            nc.sync.dma_start(out=gam[:], in_=gamma[:])
            nc.sync.dma_start(out=bet[:], in_=beta[:])
            epst = singles.tile([C, 1], F32)
            nc.vector.memset(epst[:], cfg.eps)

            # ---------- masked index fold ----------
            # idx' = mask ? idx : (n+1 | n). With bounds check (bound = n)
            # n+1 entries are skipped by the DGE and the pre-zeroed G supplies
            # the zeros; in zero_row mode they read the zero row n directly.
            idx_sb = singles.tile([128, cfg.nsub, KP], I32)
            nc.vector.memset(idx_sb[:], cfg.n if cfg.zero_row else cfg.n + 1)
            with tc.tile_pool(name="idxstage", bufs=1) as stage:
                idx_raw = stage.tile([128, cfg.nsub, KP], I32)
                msk_raw = stage.tile([128, cfg.nsub, KP], I32)
                nc.sync.dma_start(out=idx_raw[:],
                                  in_=idxT[:].rearrange("p (a k) -> p a k", k=K))
                nc.sync.dma_start(out=msk_raw[:],
                                  in_=maskT[:].rearrange("p (a k) -> p a k", k=K))
                nc.vector.copy_predicated(
                    out=idx_sb[:], mask=msk_raw[:], data=idx_raw[:]
                )

            conv_sb = singles.tile([C, cfg.shard_pad], BF16)
            stats_s = singles.tile([C, cfg.nt], F32)
            stats_q = singles.tile([C, cfg.nt], F32)

            # ---------- main conv loop ----------
            # HW constraint (probed): one indirect DMA consumes exactly one
            # row offset per partition -- 128 descriptors per instruction.
            bc_reg = None if cfg.zero_row else nc.gpsimd.to_reg(cfg.n)
            center_k = K // 2
            # Pre-zero G buffers AHEAD of use: the memset for tile t+lead is
            # emitted before tile t's PSUM->SBUF copies so it never queues
            # behind them in the DVE FIFO (which would couple the Pool gather
            # stream to PE transpose latency every tile).
            lead = 2
            gq = []
            for i in range(min(lead, cfg.nt)):
                Gn = gpool.tile([128, AT, KP, C], BF16, tag=f"g{i}")
                if not cfg.zero_row:
                    nc.vector.memset(Gn[:], 0.0)
                gq.append(Gn)
            for t in range(cfg.nt):
                G = gq.pop(0)
                if t + lead < cfg.nt:
                    Gn = gpool.tile([128, AT, KP, C], BF16, tag=f"g{(t + lead) % (lead + 2)}")
                    if not cfg.zero_row:
                        nc.vector.memset(Gn[:], 0.0)
                    gq.append(Gn)
                nc.sync.dma_start(
                    out=G[:, :, center_k, :],
                    in_=center[t * TR:(t + 1) * TR, :].rearrange(
                        "(s p) c -> p s c", p=128),
                )
                for s in range(AT):
                    a = t * AT + s
                    for k in range(KP):
                        if k == center_k:
                            continue
                        # one queue per tile (blocked): consecutive Pool
                        # instructions share the SWDGE ring context; drain
                        # still overlaps across tiles on rotating queues
                        q = t % cfg.n_queues
                        _indirect_gather_q(
                            nc,
                            out_ap=G[:, s, k, :],
                            in_ap=feats[:],
                            offset_ap=idx_sb[:, a, k:k + 1],
                            queue=f"qPoolDynamic{q or ''}",
                            bounds_reg=bc_reg,
                        )

                acc = pacc.tile([C, TR], F32)
                for j in range(cfg.npair):
                    single = (j == cfg.npair - 1) and (K % 2 == 1)
                    np_ = C if single else 2 * C
                    ptr = trp.tile([128, TR], BF16)
                    for s in range(AT):
                        nc.tensor.transpose(
                            out=ptr[:np_, s * 128:(s + 1) * 128],
                            in_=G[:, s, 2 * j:2 * j + (1 if single else 2), :],
                            identity=ident[:],
                        )
                    rhs = rhsp.tile([128, TR], BF16)
                    if cfg.act_copies and (j % 2 == 1):
                        nc.scalar.activation(
                            out=rhs[:np_, :], in_=ptr[:np_, :],
                            func=mybir.ActivationFunctionType.Copy,
                        )
                    else:
                        nc.vector.tensor_copy(out=rhs[:np_, :], in_=ptr[:np_, :])
                    nc.tensor.matmul(
                        out=acc[:],
                        lhsT=w_sb[:np_, j * C:(j + 1) * C],
                        rhs=rhs[:np_, :],
                        start=(j == 0),
                        stop=(j == cfg.npair - 1),
                    )

                # partial BN stats (all on DVE: keep ACT's function table on
                # Copy all run) + conv store
                nc.vector.reduce_sum(
                    out=stats_s[:, t:t + 1], in_=acc[:], axis=mybir.AxisListType.X
                )
                cs = conv_sb[:, t * TR:(t + 1) * TR]
                nc.vector.tensor_copy(out=cs, in_=acc[:])
                # sumsq from the SBUF bf16 copy (walrus allows only one PSUM
                # input per DVE op); consistent with the normalized values
                sq = small.tile([C, TR], F32)
                nc.vector.tensor_tensor(out=sq[:], in0=cs, in1=cs,
                                        op=mybir.AluOpType.mult)
                nc.vector.reduce_sum(
                    out=stats_q[:, t:t + 1], in_=sq[:], axis=mybir.AxisListType.X
                )

            # ---------- global BN stats (AllReduce) ----------
            sums = small.tile([C, 2], F32)
            nc.vector.reduce_sum(out=sums[:, 0:1], in_=stats_s[:], axis=mybir.AxisListType.X)
            nc.vector.reduce_sum(out=sums[:, 1:2], in_=stats_q[:], axis=mybir.AxisListType.X)
            cc_in = dram.tile([C, 2], F32)
            cc_out = dram.tile([C, 2], F32)
            nc.sync.dma_start(out=cc_in[:], in_=sums[:])
            nc.gpsimd.collective_compute(
                "AllReduce",
                mybir.AluOpType.add,
                replica_groups=[list(range(cfg.n_cores))],
                ins=[cc_in.opt()],
                outs=[cc_out.opt()],
            )
            gsum = small.tile([C, 2], F32)
            nc.sync.dma_start(out=gsum[:], in_=cc_out[:])

            mean = small.tile([C, 1], F32)
            ex2 = small.tile([C, 1], F32)
            nc.scalar.mul(out=mean[:], in_=gsum[:, 0:1], mul=1.0 / cfg.n)
            nc.scalar.mul(out=ex2[:], in_=gsum[:, 1:2], mul=1.0 / cfg.n)
            var = small.tile([C, 1], F32)
            nc.vector.tensor_tensor(out=var[:], in0=mean[:], in1=mean[:],
                                    op=mybir.AluOpType.mult)
            nc.vector.tensor_tensor(out=var[:], in0=ex2[:], in1=var[:],
                                    op=mybir.AluOpType.subtract)
            rstd = small.tile([C, 1], F32)
            nc.scalar.activation(out=rstd[:], in_=var[:],
                                 func=mybir.ActivationFunctionType.Sqrt,
                                 bias=epst[:])
            nc.vector.reciprocal(out=rstd[:], in_=rstd[:])
            scl = small.tile([C, 1], F32)
            nc.vector.tensor_tensor(out=scl[:], in0=gam[:], in1=rstd[:],
                                    op=mybir.AluOpType.mult)
            sht = small.tile([C, 1], F32)
            nc.vector.tensor_tensor(out=sht[:], in0=mean[:], in1=scl[:],
                                    op=mybir.AluOpType.mult)
            nc.vector.tensor_tensor(out=sht[:], in0=bet[:], in1=sht[:],
                                    op=mybir.AluOpType.subtract)

            # ---------- normalize + ReLU + transpose back + store ----------
            for t in range(cfg.nt):
                nb = rhsp.tile([C, TR], F32, tag="norm")
                nc.scalar.activation(
                    out=nb[:], in_=conv_sb[:, t * TR:(t + 1) * TR],
                    func=mybir.ActivationFunctionType.Relu,
                    bias=sht[:], scale=scl[:],
                )
                po = pout.tile([128, AT * C], F32)
                for s in range(AT):
                    nc.tensor.transpose(
                        out=po[:, s * C:(s + 1) * C],
                        in_=nb[:, s * 128:(s + 1) * 128],
                        identity=identf[:],
                    )
                ob = outsb.tile([128, AT * C], F32)
                nc.vector.tensor_copy(out=ob[:], in_=po[:])
                nc.sync.dma_start(
                    out=outp[t * TR:(t + 1) * TR, :].rearrange(
                        "(s p) c -> p s c", p=128
                    ),
                    in_=ob[:].rearrange("p (s c) -> p s c", c=C),
                )

    nc.compile()
    return nc


def make_in_maps(cfg: Cfg, feats, W, gamma, beta, nbr_idx, mask):
    feats_p = np.concatenate(
        [np.asarray(feats, np.float32),
         np.zeros((1, cfg.c), np.float32)], axis=0
    ).astype(BF16_NP)
    wflat = np.ascontiguousarray(
        np.asarray(W, np.float32).reshape(cfg.k * cfg.c, cfg.c)
    ).astype(BF16_NP)
    gam = np.ascontiguousarray(np.asarray(gamma, np.float32).reshape(cfg.c, 1))
    bet = np.ascontiguousarray(np.asarray(beta, np.float32).reshape(cfg.c, 1))
    nbr_idx = np.asarray(nbr_idx, np.int32)
    mask = np.asarray(mask, np.int32)
    pad = cfg.shard_pad - cfg.shard
    in_maps = []
    for core in range(cfg.n_cores):
        sl = slice(core * cfg.shard, (core + 1) * cfg.shard)
        idx_s = np.concatenate(
            [nbr_idx[:, sl], np.zeros((cfg.k, pad), np.int32)], axis=1)
        msk_s = np.concatenate(
            [mask[:, sl], np.zeros((cfg.k, pad), np.int32)], axis=1)
        # [k, nsub, 128] -> [128, nsub, k]
        idxT = np.ascontiguousarray(
            idx_s.reshape(cfg.k, cfg.nsub, 128).transpose(2, 1, 0)
        ).reshape(128, cfg.nsub * cfg.k)
        mskT = np.ascontiguousarray(
            msk_s.reshape(cfg.k, cfg.nsub, 128).transpose(2, 1, 0)
        ).reshape(128, cfg.nsub * cfg.k)
        centr = np.concatenate(
            [feats_p[core * cfg.shard:(core + 1) * cfg.shard],
             np.zeros((pad, cfg.c), BF16_NP)], axis=0)
        in_maps.append({
            "feats": feats_p, "wflat": wflat, "gamma": gam, "beta": bet,
            "idxT": idxT, "maskT": mskT, "center": centr,
        })
    return in_maps


_CACHE = {}


def _get_nc(cfg: Cfg):
    key = (cfg.n, cfg.c, cfg.k, cfg.n_cores, cfg.tile_rows, cfg.n_queues,
           cfg.act_copies, cfg.zero_row)
    if key not in _CACHE:
        _CACHE[key] = build_kernel(cfg)
    return _CACHE[key]


def run_hw(cfg: Cfg, inputs, trace=False):
    nc = _get_nc(cfg)
    in_maps = make_in_maps(cfg, **inputs)
    res = run_bass_kernel_spmd(
        nc, in_maps, core_ids=list(range(cfg.n_cores)), trace=trace
    )
    out = np.concatenate(
        [res.results[c]["out"][: cfg.shard] for c in range(cfg.n_cores)], axis=0
    )
    return np.ascontiguousarray(out, dtype=np.float32), res


def kernel(feats, W, gamma, beta, nbr_idx, mask):
    cfg = Cfg(n=feats.shape[0], c=feats.shape[1], k=W.shape[0], act_copies=False)
    out, _ = run_hw(cfg, dict(feats=feats, W=W, gamma=gamma, beta=beta,
                              nbr_idx=nbr_idx, mask=mask))
    return out


# revision 40
# speedup vs baseline: 1.0084x; 1.0006x over previous
"""Trainium2 Bass kernel for nn_BasicConvolutionBlock (gather-GEMM sparse conv + BN + ReLU).

Math (see reference): for each of K=27 kernel offsets,
    conv += (feats[nbr_idx[k]] * mask[k,:,None]) @ W[k]
then train-mode BatchNorm over the N axis (global mean/var per channel) + ReLU.

Distribution: voxel dim N sharded over 8 cores (data parallel). feats table and
weights replicated to every core; each core gathers its shard's neighbors
locally via indirect DMA. BatchNorm stats are all-reduced across cores.

Perf notes (from HW traces): the bottleneck is the Pool/SWDGE engine --
each indirect gather instruction carries exactly ONE row offset per
partition (128 descriptors; multi-offset encodings are unimplemented in
the DGE: indirection is pinned to the source's slowest dim and walrus
only lowers the 2-dim dest form -- HW-probed), and costs ~1.16us of
serialized descriptor-generation time. 196 subtiles x 26 non-center
offsets = 5096 instructions ~= 5.9ms Pool time; everything else hides
under it. Optimizations vs the first working version (9.0ms -> ~6ms):
  - bf16 pipeline (host-converted feats/weights): halves gather bytes,
    G memset cost (DVE 2x), PE transpose cycles (1 vs 2 cyc/row)
  - center k-plane (identity map, contiguous) fetched via HWDGE (sync
    engine), off the Pool engine; collective staging DMAs also on sync
  - masked taps OOB-skipped via bounds reg (cheaper in SWDGE gen + SDMA
    than gathering a zero row: dummies beat real descriptors, measured)
  - G buffers pre-zeroed 2 tiles AHEAD so the gather stream never waits
    on the DVE FIFO (memset would otherwise queue behind PSUM->SBUF
    copies that wait on PE transposes, coupling Pool to PE each tile)
  - PSUM->SBUF rhs copies all on DVE (act_copies=False): dropping ACT
    from the main loop cut cross-engine sem traffic, ~0.5% measured
Accumulation stays f32 in PSUM; BN stats are computed in f32.

Per-core pipeline:
  1. masked index fold: idx' = mask ? idx : N+1  (> bounds -> skipped, G
     pre-zeroed so masked taps contribute 0)
  2. per 512-row tile: 4x26 per-(subtile,k) indirect gathers into
     [128p, 4a, 27k, 64c] bf16 (per-tile queue blocks); per k-pair:
     PE-transpose subtiles to [128kc, 512pt] PSUM, DVE/ACT copy to SBUF
     bf16, PE matmul accumulating into PSUM [64, 512]
  3. per-tile partial stats (sum / sumsq on DVE); conv kept in SBUF bf16
  4. AllReduce [64,2] stats -> scale/shift; ACT fused affine+ReLU;
     PE transpose back to row-major; DMA out.
"""

import os
import sys

sys.path.insert(0, "/opt/trn_rl_repo")

import numpy as np
import ml_dtypes


def _install_ntff_hook_module():
    """Provide antenv.axon_hooks (NTFF profiling under axon) if the image
    lacks it, so run_bass_kernel_spmd(trace=True) can report exec_time_ns."""
    import importlib
    try:
        importlib.import_module("antenv.axon_hooks")
        return
    except ImportError:
        pass
    import contextlib
    import ctypes
    import types

    so_path = "/opt/axon/libaxon_pjrt.so"
    mod = types.ModuleType("antenv.axon_hooks")
    state = {"hook": None, "tried": False}

    def set_axon_ntff_profile_hook(hook):
        state["hook"] = hook

    def _build_hook():
        if not os.path.exists(so_path):
            return None
        lib = ctypes.CDLL(so_path)
        if not hasattr(lib, "axon_start_nrt_profile"):
            return None
        lib.axon_start_nrt_profile.argtypes = [
            ctypes.POINTER(ctypes.c_int64), ctypes.c_size_t]
        lib.axon_start_nrt_profile.restype = ctypes.c_int64
        lib.axon_stop_nrt_profile.argtypes = [ctypes.c_char_p]
        lib.axon_stop_nrt_profile.restype = ctypes.c_int64

        @contextlib.contextmanager
        def _hook(output_dir, device_ids):
            import jax
            jax.devices()
            if device_ids:
                ids = (ctypes.c_int64 * len(device_ids))(*device_ids)
                rc = lib.axon_start_nrt_profile(ids, len(device_ids))
            else:
                rc = lib.axon_start_nrt_profile(None, 0)
            if rc != 0:
                raise RuntimeError(f"axon_start_nrt_profile rc={rc}")
            try:
                yield
            finally:
                n = lib.axon_stop_nrt_profile(str(output_dir).encode())
                print(f"ntff profile: {n} file(s) -> {output_dir}",
                      file=sys.stderr)

        return _hook

    def get_axon_ntff_profile_hook():
        if state["hook"] is None and not state["tried"]:
            state["tried"] = True
            state["hook"] = _build_hook()
        return state["hook"]

    mod.set_axon_ntff_profile_hook = set_axon_ntff_profile_hook
    mod.get_axon_ntff_profile_hook = get_axon_ntff_profile_hook
    sys.modules["antenv.axon_hooks"] = mod


_install_ntff_hook_module()

import concourse.bass as bass
import concourse.bacc as bacc
import concourse.tile as tile
from concourse import mybir
from concourse.bass_utils import run_bass_kernel_spmd
from concourse.masks import make_identity

F32 = mybir.dt.float32
BF16 = mybir.dt.bfloat16
I32 = mybir.dt.int32
BF16_NP = np.dtype(ml_dtypes.bfloat16)


def _indirect_gather_q(nc, out_ap, in_ap, offset_ap, queue: str,
                       bounds_reg=None):
    """bass.indirect_dma_start (gather form) with a selectable SWDGE queue and
    an optional pre-made bounds register (indices > bound are skipped).

    The offset AP may carry many offsets per partition. The DGE pins the
    indirection dimension to the SOURCE tensor's slowest dim and fetches a
    new index each time the lockstep iteration crosses it, so the AP
    contract for an M-offsets-per-partition gather is:
      src:    [M, elem]   (outer num_elem = per-channel index count; its
                           stride is discarded -- the fetched index * coef
                           addresses the row)
      dest:   [128, M, elem]  (NOT flattened: a merged free dim becomes one
                           index + M*elem contiguous bytes)
      offset: [128, M] int32 (4B index walk per channel)
    This amortizes the ~1us SWDGE fixed cost over M*128 descriptors."""
    gp = nc.gpsimd
    table_rows = in_ap.shape[0]
    if len(out_ap.shape) > 2:
        m_per_ch = 1
        for s in out_ap.shape[1:-1]:
            m_per_ch *= s
        out_ap = out_ap.rearrange(
            f"p {' '.join(chr(97 + i) for i in range(len(out_ap.shape) - 2))} c"
            f" -> p ({' '.join(chr(97 + i) for i in range(len(out_ap.shape) - 2))}) c"
        )
        assert out_ap.shape == (128, m_per_ch, in_ap.shape[-1])
    offset_ap = offset_ap.opt()
    out_l = gp.lower_ap_dma(out_ap, for_indirect_dma=True)
    in_l = gp.lower_ap_dma(in_ap, for_indirect_dma=True)
    assert len(in_l) == 1 and len(out_l) == 1
    off_l = gp.lower_ap_dma(offset_ap)
    assert len(off_l) == 1
    in_l.append(off_l[0])
    ap_shape = in_ap.shape
    coef = 1
    for i in range(1, len(ap_shape)):
        coef *= ap_shape[i]
    in_l[0].dynamic_ap_info = mybir.DynamicAccessPatternInfo(
        c=0,
        actual_ap=out_ap.ap,
        indirect_dim_max_index=table_rows,
        offset_expr=[
            mybir.DynamicAccessPatternOffsetExpr(
                coef=coef,
                aff_expr=mybir.DynamicAccessPatternOffsetExprAffExpr(
                    kind="IndirectArgId", arg_id=1,
                ),
            )
        ],
    )
    if bounds_reg is not None:
        in_l = in_l + [gp.lower_val_access(bounds_reg)]
    return gp.add_instruction(
        mybir.InstDMACopy(
            name=nc.get_next_instruction_name(),
            queue=queue,
            mode="Copy",
            ins=in_l,
            outs=out_l,
            oob_is_err=False,
            cce_op=mybir.AluOpType.bypass,
        )
    )


class Cfg:
    def __init__(self, n=200000, c=64, k=27, n_cores=8, tile_rows=512,
                 eps=1e-5, n_queues=4, act_copies=True, zero_row=False):
        # zero_row: masked taps read the table's zero row (row n) instead of
        # being OOB-skipped -- drops the bounds check and the G memset at the
        # cost of extra (DRAM-row-hot) gather traffic.
        self.zero_row = zero_row
        assert n % n_cores == 0
        self.n, self.c, self.k, self.n_cores = n, c, k, n_cores
        self.eps = eps
        self.shard = n // n_cores
        self.nsub = (self.shard + 127) // 128          # 128-row subtiles
        self.shard_pad = self.nsub * 128
        self.tile_rows = tile_rows                     # rows per PSUM tile
        self.a_per_tile = tile_rows // 128             # subtiles per tile
        assert self.nsub % self.a_per_tile == 0
        self.nt = self.shard_pad // tile_rows          # tiles per core
        self.npair = (k + 1) // 2                      # last pair may be single
        self.n_queues = n_queues
        self.act_copies = act_copies                   # PSUM->SBUF copies split DVE/ACT
        self.table_rows = n + 1                        # + zero row (unused; bounds pad)


def build_kernel(cfg: Cfg):
    nc = bacc.Bacc("TRN2", target_bir_lowering=False, debug=False,
                   num_devices=cfg.n_cores, num_swdge_queues=cfg.n_queues)
    C, K = cfg.c, cfg.k
    TR, AT = cfg.tile_rows, cfg.a_per_tile
    KP = K

    feats = nc.dram_tensor("feats", [cfg.table_rows, C], BF16, kind="ExternalInput")
    wflat = nc.dram_tensor("wflat", [K * C, C], BF16, kind="ExternalInput")
    gamma = nc.dram_tensor("gamma", [C, 1], F32, kind="ExternalInput")
    beta = nc.dram_tensor("beta", [C, 1], F32, kind="ExternalInput")
    # host-transposed indices/mask: [128, nsub, k] with (p, a, k) = idx[k, a*128+p]
    idxT = nc.dram_tensor("idxT", [128, cfg.nsub * K], I32, kind="ExternalInput")
    maskT = nc.dram_tensor("maskT", [128, cfg.nsub * K], I32, kind="ExternalInput")
    # center k-plane of the gather is the identity map over the core's own
    # shard -- a contiguous read that HWDGE (sync engine) can do, keeping
    # those 196 transfers off the serialized Pool/SWDGE engine.
    center = nc.dram_tensor("center", [cfg.shard_pad, C], BF16, kind="ExternalInput")
    outp = nc.dram_tensor("out", [cfg.shard_pad, C], F32, kind="ExternalOutput")

    with tile.TileContext(nc) as tc:
        with (
            tc.tile_pool(name="singles", bufs=1) as singles,
            tc.tile_pool(name="gpool", bufs=1) as gpool,
            tc.tile_pool(name="trp", bufs=3, space="PSUM") as trp,
            tc.tile_pool(name="rhsp", bufs=6) as rhsp,
            tc.tile_pool(name="pacc", bufs=2, space="PSUM") as pacc,
            tc.tile_pool(name="pout", bufs=2, space="PSUM") as pout,
            tc.tile_pool(name="outsb", bufs=3) as outsb,
            tc.tile_pool(name="small", bufs=4) as small,
            tc.tile_pool(name="dram", bufs=1, space="DRAM") as dram,
        ):
            # ---------- constants ----------
            ident = singles.tile([128, 128], BF16)
            make_identity(nc, ident[:])
            identf = singles.tile([C, C], F32)
            make_identity(nc, identf[:])

            w_sb = singles.tile([128, cfg.npair * C], BF16)
            npair_full = K // 2  # pairs with both k's real
            nc.vector.memset(w_sb[:], 0.0)
            nc.sync.dma_start(
                out=w_sb[:, : npair_full * C].rearrange("p (j c) -> p j c", j=npair_full),
                in_=wflat[: npair_full * 128, :].rearrange("(j p) c -> p j c", p=128),
            )
            if K % 2:
                # trailing single k in the top 64 partitions of the last slot
                nc.sync.dma_start(
                    out=w_sb[:C, npair_full * C:(npair_full + 1) * C],
                    in_=wflat[(K - 1) * C: K * C, :],
                )

            gam = singles.tile([C, 1], F32)
            bet = singles.tile([C, 1], F32)
            nc.sync.dma_start(out=gam[:], in_=gamma[:])
            nc.sync.dma_start(out=bet[:], in_=beta[:])
            epst = singles.tile([C, 1], F32)
            nc.vector.memset(epst[:], cfg.eps)

            # ---------- masked index fold ----------
            # idx' = mask ? idx : (n+1 | n). With bounds check (bound = n)
            # n+1 entries are skipped by the DGE and the pre-zeroed G supplies
            # the zeros; in zero_row mode they read the zero row n directly.
            idx_sb = singles.tile([128, cfg.nsub, KP], I32)
            nc.vector.memset(idx_sb[:], cfg.n if cfg.zero_row else cfg.n + 1)
            with tc.tile_pool(name="idxstage", bufs=1) as stage:
                idx_raw = stage.tile([128, cfg.nsub, KP], I32)
                msk_raw = stage.tile([128, cfg.nsub, KP], I32)
                nc.sync.dma_start(out=idx_raw[:],
                                  in_=idxT[:].rearrange("p (a k) -> p a k", k=K))
                nc.sync.dma_start(out=msk_raw[:],
                                  in_=maskT[:].rearrange("p (a k) -> p a k", k=K))
                nc.vector.copy_predicated(
                    out=idx_sb[:], mask=msk_raw[:], data=idx_raw[:]
                )

            conv_sb = singles.tile([C, cfg.shard_pad], BF16)
            stats_s = singles.tile([C, cfg.nt], F32)
            stats_q = singles.tile([C, cfg.nt], F32)

            # ---------- main conv loop ----------
            # HW constraint (probed): one indirect DMA consumes exactly one
            # row offset per partition -- 128 descriptors per instruction.
            bc_reg = None if cfg.zero_row else nc.gpsimd.to_reg(cfg.n)
            center_k = K // 2
            # Pre-zero G buffers AHEAD of use: the memset for tile t+lead is
            # emitted before tile t's PSUM->SBUF copies so it never queues
            # behind them in the DVE FIFO (which would couple the Pool gather
            # stream to PE transpose latency every tile).
            lead = 2
            gq = []
            for i in range(min(lead, cfg.nt)):
                Gn = gpool.tile([128, AT, KP, C], BF16, tag=f"g{i}")
                if not cfg.zero_row:
                    nc.vector.memset(Gn[:], 0.0)
                gq.append(Gn)
            for t in range(cfg.nt):
                G = gq.pop(0)
                if t + lead < cfg.nt:
                    Gn = gpool.tile([128, AT, KP, C], BF16, tag=f"g{(t + lead) % (lead + 2)}")
                    if not cfg.zero_row:
                        nc.vector.memset(Gn[:], 0.0)
                    gq.append(Gn)
                nc.sync.dma_start(
                    out=G[:, :, center_k, :],
                    in_=center[t * TR:(t + 1) * TR, :].rearrange(
                        "(s p) c -> p s c", p=128),
                )
                for s in range(AT):
                    a = t * AT + s
                    for k in range(KP):
                        if k == center_k:
                            continue
                        # one queue per tile (blocked): consecutive Pool
                        # instructions share the SWDGE ring context; drain
                        # still overlaps across tiles on rotating queues
                        q = t % cfg.n_queues
                        _indirect_gather_q(
                            nc,
                            out_ap=G[:, s, k, :],
                            in_ap=feats[:],
                            offset_ap=idx_sb[:, a, k:k + 1],
                            queue=f"qPoolDynamic{q or ''}",
                            bounds_reg=bc_reg,
                        )

                acc = pacc.tile([C, TR], F32)
                for j in range(cfg.npair):
                    single = (j == cfg.npair - 1) and (K % 2 == 1)
                    np_ = C if single else 2 * C
                    ptr = trp.tile([128, TR], BF16)
                    for s in range(AT):
                        nc.tensor.transpose(
                            out=ptr[:np_, s * 128:(s + 1) * 128],
                            in_=G[:, s, 2 * j:2 * j + (1 if single else 2), :],
                            identity=ident[:],
                        )
                    rhs = rhsp.tile([128, TR], BF16)
                    if cfg.act_copies and (j % 2 == 1):
                        nc.scalar.activation(
                            out=rhs[:np_, :], in_=ptr[:np_, :],
                            func=mybir.ActivationFunctionType.Copy,
                        )
                    else:
                        nc.vector.tensor_copy(out=rhs[:np_, :], in_=ptr[:np_, :])
                    nc.tensor.matmul(
                        out=acc[:],
                        lhsT=w_sb[:np_, j * C:(j + 1) * C],
                        rhs=rhs[:np_, :],
                        start=(j == 0),
                        stop=(j == cfg.npair - 1),
                    )

                # partial BN stats (all on DVE: keep ACT's function table on
                # Copy all run) + conv store
                nc.vector.reduce_sum(
                    out=stats_s[:, t:t + 1], in_=acc[:], axis=mybir.AxisListType.X
                )
                cs = conv_sb[:, t * TR:(t + 1) * TR]
                nc.vector.tensor_copy(out=cs, in_=acc[:])
                # sumsq from the SBUF bf16 copy (walrus allows only one PSUM
                # input per DVE op); consistent with the normalized values
                sq = small.tile([C, TR], F32)
                nc.vector.tensor_tensor(out=sq[:], in0=cs, in1=cs,
                                        op=mybir.AluOpType.mult)
                nc.vector.reduce_sum(
                    out=stats_q[:, t:t + 1], in_=sq[:], axis=mybir.AxisListType.X
                )

            # ---------- global BN stats (AllReduce) ----------
            sums = small.tile([C, 2], F32)
            nc.vector.reduce_sum(out=sums[:, 0:1], in_=stats_s[:], axis=mybir.AxisListType.X)
            nc.vector.reduce_sum(out=sums[:, 1:2], in_=stats_q[:], axis=mybir.AxisListType.X)
            cc_in = dram.tile([C, 2], F32)
            cc_out = dram.tile([C, 2], F32)
            nc.sync.dma_start(out=cc_in[:], in_=sums[:])
            nc.gpsimd.collective_compute(
                "AllReduce",
                mybir.AluOpType.add,
                replica_groups=[list(range(cfg.n_cores))],
                ins=[cc_in.opt()],
                outs=[cc_out.opt()],
            )
            gsum = small.tile([C, 2], F32)
            nc.sync.dma_start(out=gsum[:], in_=cc_out[:])

            mean = small.tile([C, 1], F32)
            ex2 = small.tile([C, 1], F32)
            nc.scalar.mul(out=mean[:], in_=gsum[:, 0:1], mul=1.0 / cfg.n)
            nc.scalar.mul(out=ex2[:], in_=gsum[:, 1:2], mul=1.0 / cfg.n)
            var = small.tile([C, 1], F32)
            nc.vector.tensor_tensor(out=var[:], in0=mean[:], in1=mean[:],
                                    op=mybir.AluOpType.mult)
            nc.vector.tensor_tensor(out=var[:], in0=ex2[:], in1=var[:],
                                    op=mybir.AluOpType.subtract)
            rstd = small.tile([C, 1], F32)
            nc.scalar.activation(out=rstd[:], in_=var[:],
                                 func=mybir.ActivationFunctionType.Sqrt,
                                 bias=epst[:])
            nc.vector.reciprocal(out=rstd[:], in_=rstd[:])
            scl = small.tile([C, 1], F32)
            nc.vector.tensor_tensor(out=scl[:], in0=gam[:], in1=rstd[:],
                                    op=mybir.AluOpType.mult)
            sht = small.tile([C, 1], F32)
            nc.vector.tensor_tensor(out=sht[:], in0=mean[:], in1=scl[:],
                                    op=mybir.AluOpType.mult)
            nc.vector.tensor_tensor(out=sht[:], in0=bet[:], in1=sht[:],
                                    op=mybir.AluOpType.subtract)

            # ---------- normalize + ReLU + transpose back + store ----------
            for t in range(cfg.nt):
                nb = rhsp.tile([C, TR], F32, tag="norm")
                nc.scalar.activation(
                    out=nb[:], in_=conv_sb[:, t * TR:(t + 1) * TR],
                    func=mybir.ActivationFunctionType.Relu,
                    bias=sht[:], scale=scl[:],
                )
                po = pout.tile([128, AT * C], F32)
                for s in range(AT):
                    nc.tensor.transpose(
                        out=po[:, s * C:(s + 1) * C],
                        in_=nb[:, s * 128:(s + 1) * 128],
                        identity=identf[:],
                    )
                ob = outsb.tile([128, AT * C], F32)
                nc.vector.tensor_copy(out=ob[:], in_=po[:])
                nc.sync.dma_start(
                    out=outp[t * TR:(t + 1) * TR, :].rearrange(
                        "(s p) c -> p s c", p=128
                    ),
                    in_=ob[:].rearrange("p (s c) -> p s c", c=C),
                )

    nc.compile()
    return nc


def make_in_maps(cfg: Cfg, feats, W, gamma, beta, nbr_idx, mask):
    feats_p = np.concatenate(
        [np.asarray(feats, np.float32),
         np.zeros((1, cfg.c), np.float32)], axis=0
    ).astype(BF16_NP)
    wflat = np.ascontiguousarray(
        np.asarray(W, np.float32).reshape(cfg.k * cfg.c, cfg.c)
    ).astype(BF16_NP)
    gam = np.ascontiguousarray(np.asarray(gamma, np.float32).reshape(cfg.c, 1))
    bet = np.ascontiguousarray(np.asarray(beta, np.float32).reshape(cfg.c, 1))
    nbr_idx = np.asarray(nbr_idx, np.int32)
    mask = np.asarray(mask, np.int32)
    pad = cfg.shard_pad - cfg.shard
    in_maps = []
    for core in range(cfg.n_cores):
        sl = slice(core * cfg.shard, (core + 1) * cfg.shard)
        idx_s = np.concatenate(
            [nbr_idx[:, sl], np.zeros((cfg.k, pad), np.int32)], axis=1)
        msk_s = np.concatenate(
            [mask[:, sl], np.zeros((cfg.k, pad), np.int32)], axis=1)
        # [k, nsub, 128] -> [128, nsub, k]
        idxT = np.ascontiguousarray(
            idx_s.reshape(cfg.k, cfg.nsub, 128).transpose(2, 1, 0)
        ).reshape(128, cfg.nsub * cfg.k)
        mskT = np.ascontiguousarray(
            msk_s.reshape(cfg.k, cfg.nsub, 128).transpose(2, 1, 0)
        ).reshape(128, cfg.nsub * cfg.k)
        centr = np.concatenate(
            [feats_p[core * cfg.shard:(core + 1) * cfg.shard],
             np.zeros((pad, cfg.c), BF16_NP)], axis=0)
        in_maps.append({
            "feats": feats_p, "wflat": wflat, "gamma": gam, "beta": bet,
            "idxT": idxT, "maskT": mskT, "center": centr,
        })
    return in_maps


_CACHE = {}


def _get_nc(cfg: Cfg):
    key = (cfg.n, cfg.c, cfg.k, cfg.n_cores, cfg.tile_rows, cfg.n_queues,
           cfg.act_copies, cfg.zero_row)
    if key not in _CACHE:
        _CACHE[key] = build_kernel(cfg)
    return _CACHE[key]


def run_hw(cfg: Cfg, inputs, trace=False):
    nc = _get_nc(cfg)
    in_maps = make_in_maps(cfg, **inputs)
    res = run_bass_kernel_spmd(
        nc, in_maps, core_ids=list(range(cfg.n_cores)), trace=trace
    )
    out = np.concatenate(
        [res.results[c]["out"][: cfg.shard] for c in range(cfg.n_cores)], axis=0
    )
    return np.ascontiguousarray(out, dtype=np.float32), res


def kernel(feats, W, gamma, beta, nbr_idx, mask):
    cfg = Cfg(n=feats.shape[0], c=feats.shape[1], k=W.shape[0], act_copies=False)
    out, _ = run_hw(cfg, dict(feats=feats, W=W, gamma=gamma, beta=beta,
                              nbr_idx=nbr_idx, mask=mask))
    return out
